# revision 44
# baseline (speedup 1.0000x reference)
"""CRF decoder loss kernel for Trainium2 (8 NeuronCores, data-parallel over batch).

Algorithm (mathematically identical to the reference):
  The reference computes mean_b(Zp - score) where Zp is the CRF partition
  function of log_softmax(enc@W+b) and score is the gold-path score. Writing
  logits = R - logZ (R the raw projection scores, logZ the log-softmax
  normalizer), the normalizer cancels between Zp and score, so no softmax is
  ever needed. With a constant shift kappa for range control, the forward
  recursion runs in LINEAR space:

      P_0 = exp(start) * G_0,     P_t = (P_{t-1} @ exp(T)) * G_t,
      G_t = exp(R_t - kappa)                                  (all [B, V])

  loss_b = log(sum_j P_{len_b-1}[b,j] * exp(end_j))           <- S, device
           - sum_{t<len_b} (R[t,b,tgt_{t,b}] - kappa)         <- host (tiny)
           - (start[tgt_0] + sum T[tgt,tgt'] + end[tgt_last]) <- host (tiny)

Device work per core (batch shard of 32, v-major layouts):
  - projection: R^T = W^T @ encT into PSUM (bf16 matmuls, fp32 accum),
    ACT evicts G^T = exp(R^T + (b - kappa)) as bf16.
  - scan: state P^T [v, 32] bf16 in a 32-slot ring; per step 4 matmuls with
    the four 128x128 blocks of exp(T) and ONE fused DVE multiply [128,2,32]
    by G_t^T — a single semaphore round-trip per step. Projection and
    S-extraction work is queued as filler ops between scan steps; the list
    scheduler absorbs it into the scan's semaphore-wait windows.
  - S extraction: every 16 steps batched matmuls with exp(end) over the ring
    yield S_t[b] for all (t, b); host picks t = len_b - 1.
"""

import numpy as np
import ml_dtypes

import concourse.bacc as bacc
import concourse.tile as tile
from concourse import mybir
from concourse.bass_utils import run_bass_kernel_spmd

bf16 = ml_dtypes.bfloat16
f32 = mybir.dt.float32
bf16_t = mybir.dt.bfloat16

S, B, H, V = 512, 256, 512, 256
NCORES = 8
BC = B // NCORES            # 32 batch per core
ROWS = S * BC               # 16384 rows (t-major, b-minor)
KAPPA = 6.05
CHUNK = 512                 # projection chunk (rows)
NCHUNK = ROWS // CHUNK      # 32
SBLK = 16                   # scan steps per S-extraction block
RING = 32                   # state ring slots
LEAD = 1                    # projection chunks ahead of the scan

_nc_cache = None


def _build():
    nc = bacc.Bacc("TRN2", debug=False)

    encT = nc.dram_tensor("encT", [128, NCHUNK, 4, CHUNK], bf16_t, kind="ExternalInput")
    # all bf16 constants in one blob (one DMA): 8 W blocks | 4 expT blocks
    # | expEnd (2 cols); all f32 constants in another: bias (2) | expStart (2)
    cbf = nc.dram_tensor("cbf", [128, 1538], bf16_t, kind="ExternalInput")
    cf32 = nc.dram_tensor("cf32", [128, 4], f32, kind="ExternalInput")

    s_out = nc.dram_tensor("s_out", [1, ROWS], f32, kind="ExternalOutput")

    with tile.TileContext(nc) as tc:
        with (
            tc.tile_pool(name="consts", bufs=1) as consts,
            tc.tile_pool(name="encp", bufs=3) as encp,
            tc.tile_pool(name="gpool", bufs=1) as gpool,
            tc.tile_pool(name="proj_ps", bufs=3, space="PSUM") as proj_ps,
            tc.tile_pool(name="scan_ps", bufs=2, space="PSUM") as scan_ps,
            tc.tile_pool(name="s_ps", bufs=2, space="PSUM") as s_ps,
        ):
            cb = consts.tile([128, 1538], bf16_t)
            cf = consts.tile([128, 4], f32)
            s_sb = consts.tile([1, ROWS], f32)
            ring = consts.tile([128, RING, 2, BC], bf16_t)

            nc.sync.dma_start(out=cb[:], in_=cbf[:])
            nc.sync.dma_start(out=cf[:], in_=cf32[:])

            def w_blk(i):          # W block i of 8
                return cb[:, i * 128:(i + 1) * 128]

            def expT_blk(i):       # expT block i of 4
                return cb[:, 1024 + i * 128:1024 + (i + 1) * 128]

            def expEnd_col(ih):
                return cb[:, 1536 + ih:1537 + ih]

            def bias_col(vh):
                return cf[:, vh:vh + 1]

            def expStart_col(ih):
                return cf[:, 2 + ih:3 + ih]

            gtiles = []
            fillers = []

            def push_proj_chunk(c):
                """DMA now; 8 matmuls + 2 exp-evictions as fillers."""
                et = encp.tile([128, 4, CHUNK], bf16_t, name="et", tag="enc")
                nc.sync.dma_start(out=et[:], in_=encT[:, c, :, :])
                g = gpool.tile([128, 2, CHUNK], bf16_t, name=f"g{c}", tag=f"g{c}")
                gtiles.append(g)
                ps_box = [None, None]

                def mk_mm(vh, ht):
                    def mm():
                        if ht == 0:
                            ps_box[vh] = proj_ps.tile(
                                [128, CHUNK], f32, name="pps", tag="pps")
                        nc.tensor.matmul(
                            ps_box[vh][:],
                            lhsT=w_blk(ht * 2 + vh),
                            rhs=et[:, ht, :],
                            start=(ht == 0),
                            stop=(ht == 3),
                        )
                    return mm

                def mk_exp(vh):
                    def ex():
                        nc.scalar.activation(
                            g[:, vh, :], ps_box[vh][:],
                            mybir.ActivationFunctionType.Exp,
                            bias=bias_col(vh), scale=1.0,
                        )
                    return ex

                for vh in range(2):
                    for ht in range(4):
                        fillers.append(mk_mm(vh, ht))
                    fillers.append(mk_exp(vh))

            def push_sblock(k):
                """S_t for steps in block k: 4 matmul fillers + 1 copy."""
                sp = s_ps.tile([1, SBLK * BC], f32, name="sps", tag="sps")
                s0 = (k * SBLK) % RING

                def mk_mm(half, ih):
                    def mm():
                        nc.tensor.matmul(
                            sp[0:1, half * 256:(half + 1) * 256],
                            lhsT=expEnd_col(ih),
                            rhs=ring[:, s0 + half * 8:s0 + (half + 1) * 8, ih, :],
                            start=(ih == 0),
                            stop=(ih == 1),
                        )
                    return mm

                def cp():
                    nc.scalar.copy(
                        s_sb[0:1, k * (SBLK * BC):(k + 1) * (SBLK * BC)], sp[:])

                for half in range(2):
                    for ih in range(2):
                        fillers.append(mk_mm(half, ih))
                fillers.append(cp)

            def filler_tick():
                if fillers:
                    fillers.pop(0)()

            # ------------- prologue: chunk 0 in column-quarters -------------
            # the scan's step t only needs G columns (t%16)*32..+32, so emit
            # chunk 0 as four free-128 column slices, each immediately
            # evicted by ACT: the scan launches after quarter 0 (~2us)
            # instead of after the whole chunk (~7us at cold-PE p-state)
            et0 = encp.tile([128, 4, CHUNK], bf16_t, name="et", tag="enc")
            nc.sync.dma_start(out=et0[:], in_=encT[:, 0, :, :])
            g0 = gpool.tile([128, 2, CHUNK], bf16_t, name="g0", tag="g0")
            gtiles.append(g0)
            ps0 = [proj_ps.tile([128, CHUNK], f32, name="pps", tag="pps")
                   for _ in range(2)]
            for q in range(4):
                for vh in range(2):
                    for ht in range(4):
                        nc.tensor.matmul(
                            ps0[vh][:, q * 128:(q + 1) * 128],
                            lhsT=w_blk(ht * 2 + vh),
                            rhs=et0[:, ht, q * 128:(q + 1) * 128],
                            start=(ht == 0),
                            stop=(ht == 3),
                        )
                    nc.scalar.activation(
                        g0[:, vh, q * 128:(q + 1) * 128],
                        ps0[vh][:, q * 128:(q + 1) * 128],
                        mybir.ActivationFunctionType.Exp,
                        bias=bias_col(vh), scale=1.0,
                    )
            for c in range(1, LEAD):
                push_proj_chunk(c)
            while fillers:
                filler_tick()

            # ---------------- scan ----------------
            for ih in range(2):
                nc.vector.tensor_scalar_mul(
                    ring[:, 0, ih, :],
                    in0=gtiles[0][:, ih, 0:BC],
                    scalar1=expStart_col(ih),
                )

            for t in range(1, S):
                if t % SBLK == 1:
                    blk = t // SBLK
                    if blk + LEAD < NCHUNK:
                        push_proj_chunk(blk + LEAD)
                    if blk >= 1:
                        push_sblock(blk - 1)

                gt = gtiles[t // SBLK]
                off = (t % SBLK) * BC
                ps = scan_ps.tile([128, 2, BC], f32, name="ps", tag="ps")
                for jh in range(2):
                    for ih in range(2):
                        nc.tensor.matmul(
                            ps[:, jh, :],
                            lhsT=expT_blk(ih * 2 + jh),
                            rhs=ring[:, (t - 1) % RING, ih, :],
                            start=(ih == 0),
                            stop=(ih == 1),
                        )
                # one fused DVE multiply for both j-halves: a single
                # semaphore round-trip per scan step
                nc.vector.tensor_tensor(
                    out=ring[:, t % RING, :, :],
                    in0=ps[:],
                    in1=gt[:, :, off:off + BC],
                    op=mybir.AluOpType.mult,
                )
                filler_tick()

            push_sblock(NCHUNK - 1)
            while fillers:
                filler_tick()

            nc.sync.dma_start(out=s_out[:], in_=s_sb[:])

    nc.compile()
    return nc


def _host_consts(d):
    W_ = np.asarray(d["W"], dtype=np.float32)
    b_ = np.asarray(d["b"], dtype=np.float64)
    T_ = np.asarray(d["transition"], dtype=np.float64)
    start_ = np.asarray(d["start_transition"], dtype=np.float64)
    end_ = np.asarray(d["end_transition"], dtype=np.float64)
    Wb = W_.reshape(4, 128, 2, 128).transpose(1, 0, 2, 3).reshape(128, 1024)
    expTb = np.exp(T_).reshape(2, 128, 2, 128).transpose(1, 0, 2, 3).reshape(128, 512)
    expEndT = np.exp(end_).reshape(2, 128).T
    cbf = np.ascontiguousarray(
        np.concatenate([Wb, expTb, expEndT], axis=1)).astype(bf16)
    biasT = (b_ - KAPPA).reshape(2, 128).T
    expStartT = np.exp(start_).reshape(2, 128).T
    cf32 = np.ascontiguousarray(
        np.concatenate([biasT, expStartT], axis=1)).astype(np.float32)
    return cbf, cf32


def _prep_core_inputs(core, enc_bf, cbf, cf32):
    # encT layout [h%128, chunk, h//128, row-in-chunk]; rows are t*BC + b
    b0 = core * BC
    e = enc_bf[:, b0:b0 + BC, :].transpose(2, 0, 1).reshape(4, 128, NCHUNK, CHUNK)
    e = np.ascontiguousarray(e.transpose(1, 2, 0, 3))
    return {"encT": e, "cbf": cbf, "cf32": cf32}


def kernel(enc_outs, W, b, transition, start_transition, end_transition,
           targets, lengths):
    global _nc_cache
    if _nc_cache is None:
        _nc_cache = _build()
    nc = _nc_cache

    enc = np.asarray(enc_outs, dtype=np.float32)
    W_ = np.asarray(W, dtype=np.float32)
    b_ = np.asarray(b, dtype=np.float64)
    T_ = np.asarray(transition, dtype=np.float64)
    start_ = np.asarray(start_transition, dtype=np.float64)
    end_ = np.asarray(end_transition, dtype=np.float64)
    tgt = np.asarray(targets).astype(np.int64)
    lens = np.asarray(lengths).astype(np.int64)

    cbf, cf32 = _host_consts({
        "W": W, "b": b, "transition": transition,
        "start_transition": start_transition, "end_transition": end_transition,
    })
    enc_bf = enc.astype(bf16)
    in_maps = [
        _prep_core_inputs(c, enc_bf, cbf, cf32)
        for c in range(NCORES)
    ]
    res = run_bass_kernel_spmd(nc, in_maps, list(range(NCORES))).results

    # ---------------- host epilogue (small inputs only) ----------------
    tmask = (np.arange(S)[:, None] < lens[None, :])
    trans_sum = (T_[tgt[:-1], tgt[1:]] * tmask[1:]).sum(axis=0)
    last_tgt = tgt[lens - 1, np.arange(B)]
    hostscore = start_[tgt[0]] + trans_sum + end_[last_tgt]

    # gold-path raw emission scores: R[t, b, tgt] = enc[t, b] . W[:, tgt] + b
    # (16K dot products per core; 0.1% of the device FLOPs)
    Wg = W_.T[tgt.reshape(-1)]                        # (S*B, H)
    emis_all = (np.einsum("rh,rh->r", enc.reshape(S * B, H), Wg,
                          optimize=True).reshape(S, B)
                + b_[tgt])
    emis = ((emis_all - KAPPA) * tmask).sum(axis=0)

    loss_b = np.zeros(B, dtype=np.float64)
    for c in range(NCORES):
        b0 = c * BC
        s_flat = np.asarray(res[c]["s_out"], dtype=np.float64).reshape(ROWS)
        # S col layout: (t//SBLK) * 512 + (t%SBLK) * BC + b
        s_dec = s_flat.reshape(S // SBLK, SBLK, BC)
        bl = lens[b0:b0 + BC] - 1
        blocal = np.arange(BC)
        s_end = s_dec[bl // SBLK, bl % SBLK, blocal]
        loss_b[b0:b0 + BC] = np.log(s_end) - emis[b0:b0 + BC] \
            - hostscore[b0:b0 + BC]

    return np.float32(loss_b.mean())


# revision 51
# speedup vs baseline: 1.0111x; 1.0111x over previous
"""CRF decoder loss kernel for Trainium2 (8 NeuronCores, data-parallel over batch).

Algorithm (mathematically identical to the reference):
  The reference computes mean_b(Zp - score) where Zp is the CRF partition
  function of log_softmax(enc@W+b) and score is the gold-path score. Writing
  logits = R - logZ (R the raw projection scores, logZ the log-softmax
  normalizer), the normalizer cancels between Zp and score, so no softmax is
  ever needed. With a constant shift kappa for range control, the forward
  recursion runs in LINEAR space:

      P_0 = exp(start) * G_0,     P_t = (P_{t-1} @ exp(T)) * G_t,
      G_t = exp(R_t - kappa)                                  (all [B, V])

  loss_b = log(sum_j P_{len_b-1}[b,j] * exp(end_j))           <- S, device
           - sum_{t<len_b} (R[t,b,tgt_{t,b}] - kappa)         <- host (tiny)
           - (start[tgt_0] + sum T[tgt,tgt'] + end[tgt_last]) <- host (tiny)

Device work per core (batch shard of 32, v-major layouts):
  - projection: R^T = W^T @ encT into PSUM (bf16 matmuls, fp32 accum),
    ACT evicts G^T = exp(R^T + (b - kappa)) as bf16.
  - scan: state P^T [v, 32] bf16 in a 32-slot ring; per step 4 matmuls with
    the four 128x128 blocks of exp(T) and ONE fused DVE multiply [128,2,32]
    by G_t^T — a single semaphore round-trip per step. Projection and
    S-extraction work is queued as filler ops between scan steps; the list
    scheduler absorbs it into the scan's semaphore-wait windows.
  - S extraction: every 16 steps batched matmuls with exp(end) over the ring
    yield S_t[b] for all (t, b); host picks t = len_b - 1.
"""

import numpy as np
import ml_dtypes

import concourse.bacc as bacc
import concourse.tile as tile
from concourse import mybir
from concourse.bass_utils import run_bass_kernel_spmd

bf16 = ml_dtypes.bfloat16
f32 = mybir.dt.float32
bf16_t = mybir.dt.bfloat16

S, B, H, V = 512, 256, 512, 256
NCORES = 8
BC = B // NCORES            # 32 batch per core
ROWS = S * BC               # 16384 rows (t-major, b-minor)
KAPPA = 6.05
CHUNK = 512                 # projection chunk (rows)
NCHUNK = ROWS // CHUNK      # 32
SBLK = 16                   # scan steps per S-extraction block
RING = 32                   # state ring slots
LEAD = 1                    # projection chunks ahead of the scan

_nc_cache = None


def _build():
    nc = bacc.Bacc("TRN2", debug=False)

    encT = nc.dram_tensor("encT", [128, NCHUNK, 4, CHUNK], bf16_t, kind="ExternalInput")
    # all bf16 constants in one blob (one DMA): 8 W blocks | 4 expT blocks
    # | expEnd (2 cols); all f32 constants in another: bias (2) | expStart (2)
    cbf = nc.dram_tensor("cbf", [128, 1538], bf16_t, kind="ExternalInput")
    cf32 = nc.dram_tensor("cf32", [128, 4], f32, kind="ExternalInput")
    # scan stationary exp(T) in fp8e4: halves the weight-load bytes on the
    # gated LDWEIGHTS of every scan step (accuracy headroom is ~10x: e4m3
    # rounding of exp(T)~1.0 biases the loss by O(1) vs tolerance ~31)
    expT8 = nc.dram_tensor("expT8", [128, 512], mybir.dt.float8e4,
                           kind="ExternalInput")

    s_out = nc.dram_tensor("s_out", [1, ROWS], f32, kind="ExternalOutput")

    with tile.TileContext(nc) as tc:
        with (
            tc.tile_pool(name="consts", bufs=1) as consts,
            tc.tile_pool(name="encp", bufs=3) as encp,
            tc.tile_pool(name="gpool", bufs=1) as gpool,
            tc.tile_pool(name="proj_ps", bufs=3, space="PSUM") as proj_ps,
            tc.tile_pool(name="scan_ps", bufs=2, space="PSUM") as scan_ps,
            tc.tile_pool(name="s_ps", bufs=2, space="PSUM") as s_ps,
        ):
            cb = consts.tile([128, 1538], bf16_t)
            cf = consts.tile([128, 4], f32)
            s_sb = consts.tile([1, ROWS], f32)
            ring = consts.tile([128, RING, 2, BC], bf16_t)

            e8 = consts.tile([128, 512], mybir.dt.float8e4)

            nc.sync.dma_start(out=cb[:], in_=cbf[:])
            nc.sync.dma_start(out=cf[:], in_=cf32[:])
            nc.sync.dma_start(out=e8[:], in_=expT8[:])

            def w_blk(i):          # W block i of 8
                return cb[:, i * 128:(i + 1) * 128]

            def expT_blk(i):       # expT block i of 4 (fp8e4 stationary)
                return e8[:, i * 128:(i + 1) * 128]

            def expEnd_col(ih):
                return cb[:, 1536 + ih:1537 + ih]

            def bias_col(vh):
                return cf[:, vh:vh + 1]

            def expStart_col(ih):
                return cf[:, 2 + ih:3 + ih]

            gtiles = []
            fillers = []

            def push_proj_chunk(c):
                """DMA now; 8 matmuls + 2 exp-evictions as fillers."""
                et = encp.tile([128, 4, CHUNK], bf16_t, name="et", tag="enc")
                nc.sync.dma_start(out=et[:], in_=encT[:, c, :, :])
                g = gpool.tile([128, 2, CHUNK], bf16_t, name=f"g{c}", tag=f"g{c}")
                gtiles.append(g)
                ps_box = [None, None]

                def mk_mm(vh, ht):
                    def mm():
                        if ht == 0:
                            ps_box[vh] = proj_ps.tile(
                                [128, CHUNK], f32, name="pps", tag="pps")
                        nc.tensor.matmul(
                            ps_box[vh][:],
                            lhsT=w_blk(ht * 2 + vh),
                            rhs=et[:, ht, :],
                            start=(ht == 0),
                            stop=(ht == 3),
                        )
                    return mm

                def mk_exp(vh):
                    def ex():
                        nc.scalar.activation(
                            g[:, vh, :], ps_box[vh][:],
                            mybir.ActivationFunctionType.Exp,
                            bias=bias_col(vh), scale=1.0,
                        )
                    return ex

                for vh in range(2):
                    for ht in range(4):
                        fillers.append(mk_mm(vh, ht))
                    fillers.append(mk_exp(vh))

            def push_sblock(k):
                """S_t for steps in block k: 4 matmul fillers + 1 copy."""
                sp = s_ps.tile([1, SBLK * BC], f32, name="sps", tag="sps")
                s0 = (k * SBLK) % RING

                def mk_mm(half, ih):
                    def mm():
                        nc.tensor.matmul(
                            sp[0:1, half * 256:(half + 1) * 256],
                            lhsT=expEnd_col(ih),
                            rhs=ring[:, s0 + half * 8:s0 + (half + 1) * 8, ih, :],
                            start=(ih == 0),
                            stop=(ih == 1),
                        )
                    return mm

                def cp():
                    nc.scalar.copy(
                        s_sb[0:1, k * (SBLK * BC):(k + 1) * (SBLK * BC)], sp[:])

                for half in range(2):
                    for ih in range(2):
                        fillers.append(mk_mm(half, ih))
                fillers.append(cp)

            def filler_tick():
                if fillers:
                    fillers.pop(0)()

            # ---------------- prologue: chunks 0..LEAD-1 fully ----------------
            for c in range(LEAD):
                push_proj_chunk(c)
            while fillers:
                filler_tick()

            # ---------------- scan ----------------
            for ih in range(2):
                nc.vector.tensor_scalar_mul(
                    ring[:, 0, ih, :],
                    in0=gtiles[0][:, ih, 0:BC],
                    scalar1=expStart_col(ih),
                )

            for t in range(1, S):
                if t % SBLK == 1:
                    blk = t // SBLK
                    if blk + LEAD < NCHUNK:
                        push_proj_chunk(blk + LEAD)
                    if blk >= 1:
                        push_sblock(blk - 1)

                gt = gtiles[t // SBLK]
                off = (t % SBLK) * BC
                ps = scan_ps.tile([128, 2, BC], f32, name="ps", tag="ps")
                for jh in range(2):
                    for ih in range(2):
                        nc.tensor.matmul(
                            ps[:, jh, :],
                            lhsT=expT_blk(ih * 2 + jh),
                            rhs=ring[:, (t - 1) % RING, ih, :],
                            start=(ih == 0),
                            stop=(ih == 1),
                        )
                # one fused DVE multiply for both j-halves: a single
                # semaphore round-trip per scan step
                nc.vector.tensor_tensor(
                    out=ring[:, t % RING, :, :],
                    in0=ps[:],
                    in1=gt[:, :, off:off + BC],
                    op=mybir.AluOpType.mult,
                )
                filler_tick()

            push_sblock(NCHUNK - 1)
            while fillers:
                filler_tick()

            nc.sync.dma_start(out=s_out[:], in_=s_sb[:])

    nc.compile()
    return nc


def _host_consts(d):
    W_ = np.asarray(d["W"], dtype=np.float32)
    b_ = np.asarray(d["b"], dtype=np.float64)
    T_ = np.asarray(d["transition"], dtype=np.float64)
    start_ = np.asarray(d["start_transition"], dtype=np.float64)
    end_ = np.asarray(d["end_transition"], dtype=np.float64)
    Wb = W_.reshape(4, 128, 2, 128).transpose(1, 0, 2, 3).reshape(128, 1024)
    expTb = np.exp(T_).reshape(2, 128, 2, 128).transpose(1, 0, 2, 3).reshape(128, 512)
    expEndT = np.exp(end_).reshape(2, 128).T
    cbf = np.ascontiguousarray(
        np.concatenate([Wb, expTb, expEndT], axis=1)).astype(bf16)
    biasT = (b_ - KAPPA).reshape(2, 128).T
    expStartT = np.exp(start_).reshape(2, 128).T
    cf32 = np.ascontiguousarray(
        np.concatenate([biasT, expStartT], axis=1)).astype(np.float32)
    expT8 = np.ascontiguousarray(expTb).astype(ml_dtypes.float8_e4m3fn)
    return cbf, cf32, expT8


def _prep_core_inputs(core, enc_bf, cbf, cf32, expT8):
    # encT layout [h%128, chunk, h//128, row-in-chunk]; rows are t*BC + b
    b0 = core * BC
    e = enc_bf[:, b0:b0 + BC, :].transpose(2, 0, 1).reshape(4, 128, NCHUNK, CHUNK)
    e = np.ascontiguousarray(e.transpose(1, 2, 0, 3))
    return {"encT": e, "cbf": cbf, "cf32": cf32, "expT8": expT8}


def kernel(enc_outs, W, b, transition, start_transition, end_transition,
           targets, lengths):
    global _nc_cache
    if _nc_cache is None:
        _nc_cache = _build()
    nc = _nc_cache

    enc = np.asarray(enc_outs, dtype=np.float32)
    W_ = np.asarray(W, dtype=np.float32)
    b_ = np.asarray(b, dtype=np.float64)
    T_ = np.asarray(transition, dtype=np.float64)
    start_ = np.asarray(start_transition, dtype=np.float64)
    end_ = np.asarray(end_transition, dtype=np.float64)
    tgt = np.asarray(targets).astype(np.int64)
    lens = np.asarray(lengths).astype(np.int64)

    cbf, cf32, expT8 = _host_consts({
        "W": W, "b": b, "transition": transition,
        "start_transition": start_transition, "end_transition": end_transition,
    })
    enc_bf = enc.astype(bf16)
    in_maps = [
        _prep_core_inputs(c, enc_bf, cbf, cf32, expT8)
        for c in range(NCORES)
    ]
    res = run_bass_kernel_spmd(nc, in_maps, list(range(NCORES))).results

    # ---------------- host epilogue (small inputs only) ----------------
    tmask = (np.arange(S)[:, None] < lens[None, :])
    trans_sum = (T_[tgt[:-1], tgt[1:]] * tmask[1:]).sum(axis=0)
    last_tgt = tgt[lens - 1, np.arange(B)]
    hostscore = start_[tgt[0]] + trans_sum + end_[last_tgt]

    # gold-path raw emission scores: R[t, b, tgt] = enc[t, b] . W[:, tgt] + b
    # (16K dot products per core; 0.1% of the device FLOPs)
    Wg = W_.T[tgt.reshape(-1)]                        # (S*B, H)
    emis_all = (np.einsum("rh,rh->r", enc.reshape(S * B, H), Wg,
                          optimize=True).reshape(S, B)
                + b_[tgt])
    emis = ((emis_all - KAPPA) * tmask).sum(axis=0)

    loss_b = np.zeros(B, dtype=np.float64)
    for c in range(NCORES):
        b0 = c * BC
        s_flat = np.asarray(res[c]["s_out"], dtype=np.float64).reshape(ROWS)
        # S col layout: (t//SBLK) * 512 + (t%SBLK) * BC + b
        s_dec = s_flat.reshape(S // SBLK, SBLK, BC)
        bl = lens[b0:b0 + BC] - 1
        blocal = np.arange(BC)
        s_end = s_dec[bl // SBLK, bl % SBLK, blocal]
        loss_b[b0:b0 + BC] = np.log(s_end) - emis[b0:b0 + BC] \
            - hostscore[b0:b0 + BC]

    return np.float32(loss_b.mean())
